# revision 1
# baseline (speedup 1.0000x reference)
"""Bidirectional Mamba2 layer on 8 NeuronCores.

Sharding: 8 cores = 4 batch elements x 2 directions (fw/bw). Each core runs
one full Mamba2 layer pass on one sequence; the host flips the bw sequences,
adds fw+bw results, and applies the padding mask.

Per-core kernel (sequence length L=2048, chunked SSD scan with T=128):
  1. in_proj as channel-major matmuls (fp32r), producing silu(z), pre-conv
     xBC (bf16), and dt (softplus, fp32).
  2. depthwise conv width-4 in channel-major via scalar_tensor_tensor with
     per-partition weights, then silu (+bias) -> xBC_conv (bf16).
  3. DRAM roundtrip with DMA-transpose to obtain time-major xBC per chunk.
  4. chunked scan: decay matrices from constant triangular-matrix matmuls on
     log(dA) + ACT exp; per-head Y = intra (M'' matmul) + inter (state)
     contributions accumulated in PSUM, channel-major.
  5. gated RMSNorm (partition-reduction via matmul) and out_proj (bf16).
"""

import numpy as np

D_MODEL = 512
D_STATE = 128
NH = 16
HD = 64
D_INNER = 1024
D_XBC = 1280
D_IN = 2320
L = 2048
T = 128
NCH = L // T
B_SZ = 4
EPS = 1e-5
NEG_INF = -1e30

_CACHE = {}


def _patch_drain(tile, mybir, ScopedClock):
    # workaround: this walrus build rejects >2 sem waits per instruction;
    # spread the TileContext exit-drain waits across nop instructions.
    def _drain_and_barrier(self, tick_clock, wait_clock):
        nc_ = self.nc
        probe = nc_.sync.nop()
        wait_clock.add_sem_waits(
            probe.ins, ScopedClock({None: tick_clock.global_clock})
        )
        waits = list(probe.ins.sync_info.on_wait or [])
        if probe.ins.sync_info is not None:
            probe.ins.sync_info.on_wait = waits[:1]
            rest = waits[1:]
        else:
            rest = []
        for w in rest:
            n = nc_.sync.nop()
            if n.ins.sync_info is None:
                n.ins.sync_info = mybir.SyncInfo(on_wait=[w], on_update=[])
            else:
                n.ins.sync_info.on_wait = [w]
        nc_.sync.drain()
        nc_.all_engine_barrier()
        assert self.sems is not None
        popped = nc_._tile_sem_poison_stack.pop()
        assert popped is self._sem_poison
        nc_.clear_and_free_semaphores(list(self.sems.allocated().values()))
        nc_.all_engine_barrier()

    tile.TileContext._drain_and_barrier = _drain_and_barrier


def _build_program():
    import concourse.bass as bass
    import concourse.mybir as mybir
    import concourse.tile as tile
    from concourse.vector_clock import ScopedClock

    _patch_drain(tile, mybir, ScopedClock)

    f32 = mybir.dt.float32
    f32r = mybir.dt.float32r
    bf16 = mybir.dt.bfloat16
    AF = mybir.ActivationFunctionType
    OP = mybir.AluOpType

    nc = bass.Bass("TRN2", target_bir_lowering=False, debug=False)

    # ---------------- DRAM I/O ----------------
    xT_d = nc.dram_tensor("xT", [D_MODEL, L], bf16, kind="ExternalInput")
    w_in_d = nc.dram_tensor("w_in", [D_MODEL, D_IN], bf16, kind="ExternalInput")
    w_out_d = nc.dram_tensor("w_out", [D_INNER, D_MODEL], f32, kind="ExternalInput")
    convw_d = nc.dram_tensor("convw", [128, 10, 4], f32, kind="ExternalInput")
    convb_d = nc.dram_tensor("convb", [128, 10], f32, kind="ExternalInput")
    dtb_d = nc.dram_tensor("dtb", [16, 1], f32, kind="ExternalInput")
    nae_d = nc.dram_tensor("nae", [16, 1], f32, kind="ExternalInput")  # -exp(A_log)
    dcol_d = nc.dram_tensor("dcol", [128, 8], f32, kind="ExternalInput")  # D per pair-tile
    nrmw_d = nc.dram_tensor("nrmw", [128, 8], f32, kind="ExternalInput")
    alow_d = nc.dram_tensor("alow", [128, 128], mybir.dt.bfloat16, kind="ExternalInput")
    uinc_d = nc.dram_tensor("uinc", [128, 128], mybir.dt.bfloat16, kind="ExternalInput")
    idnb_d = nc.dram_tensor("idnb", [128, 128], mybir.dt.bfloat16, kind="ExternalInput")
    idnf_d = nc.dram_tensor("idnf", [128, 128], f32, kind="ExternalInput")
    ones_d = nc.dram_tensor("ones", [128, 1], mybir.dt.bfloat16, kind="ExternalInput")
    onesrf_d = nc.dram_tensor("onesrf", [1, 128], f32, kind="ExternalInput")
    onesrb_d = nc.dram_tensor("onesrb", [1, 128], mybir.dt.bfloat16, kind="ExternalInput")
    minf4_d = nc.dram_tensor("minf4", [128, 512], mybir.dt.bfloat16, kind="ExternalInput")
    yT_d = nc.dram_tensor("yT", [D_MODEL, L], f32, kind="ExternalOutput")

    with tile.TileContext(nc) as tc:
        with (
            tc.tile_pool(name="const", bufs=1) as cpool,
            tc.tile_pool(name="dram", bufs=1, space="DRAM") as dpool,
            tc.tile_pool(name="mid", bufs=1) as mid,
        ):
            # ---------------- constants ----------------
            ALOW = cpool.tile([128, 128], bf16, tag="alow")
            nc.sync.dma_start(ALOW[:], alow_d.ap())
            UINC = cpool.tile([128, 128], bf16, tag="uinc")
            nc.sync.dma_start(UINC[:], uinc_d.ap())
            IDNB = cpool.tile([128, 128], bf16, tag="idnb")
            nc.sync.dma_start(IDNB[:], idnb_d.ap())
            IDNF = cpool.tile([128, 128], f32, tag="idnf")
            nc.sync.dma_start(IDNF[:], idnf_d.ap())
            ONEC = cpool.tile([128, 1], bf16, tag="ones")
            nc.sync.dma_start(ONEC[:], ones_d.ap())
            ONESRF = cpool.tile([1, 128], f32, tag="onesrf")
            nc.sync.dma_start(ONESRF[:], onesrf_d.ap())
            ONESRB = cpool.tile([1, 128], bf16, tag="onesrb")
            nc.sync.dma_start(ONESRB[:], onesrb_d.ap())
            CONVW = cpool.tile([128, 10, 4], f32, tag="convw")
            nc.sync.dma_start(CONVW[:], convw_d.ap())
            CONVB = cpool.tile([128, 10], f32, tag="convb")
            nc.sync.dma_start(CONVB[:], convb_d.ap())
            DTB = cpool.tile([16, 1], f32, tag="dtb")
            nc.sync.dma_start(DTB[:], dtb_d.ap())
            NAE = cpool.tile([16, 1], f32, tag="nae")
            nc.sync.dma_start(NAE[:], nae_d.ap())
            DCOL = cpool.tile([128, 8], f32, tag="dcol")
            nc.sync.dma_start(DCOL[:], dcol_d.ap())
            NRMW = cpool.tile([128, 8], f32, tag="nrmw")
            nc.sync.dma_start(NRMW[:], nrmw_d.ap())
            EPSC = cpool.tile([128, 1], f32, tag="epsc")
            nc.vector.memset(EPSC[:], EPS)
            MINF4 = cpool.tile([128, 512], bf16, tag="minf4")
            nc.sync.dma_start(MINF4[:], minf4_d.ap())

            # ---------------- small persistent tensors ----------------
            dtld = mid.tile([96, L], f32, tag="dtld")           # dt rows 0:16, logdA 16:32
            dtldT = mid.tile([128, NCH, 96], f32, tag="dtldT")  # time-major dt/logdA
            atot = mid.tile([16, 16], f32, tag="atot")          # [head, chunk]
            atotT = mid.tile([16, 16], f32, tag="atotT")        # [chunk, head]
            s_sb = [mid.tile([128, NH, HD], bf16, tag=f"s_sb{i}", name=f"s_sb{i}")
                    for i in range(2)]
            atotF = mid.tile([1, 256], f32, tag="atotF")

            rt_dram = dpool.tile([D_XBC, L], bf16)              # roundtrip buffer

            with tc.tile_pool(name="p_sz", bufs=1) as p_sz:
                sz = p_sz.tile([128, 8, L], bf16, tag="sz")     # silu(z), ch-major
                with tc.tile_pool(name="p_ysb", bufs=1) as p_ysb:
                    y_sb = p_ysb.tile([128, 8, L], bf16, tag="y_sb")

                    with tc.tile_pool(name="p_xbc", bufs=1) as p_xbc:
                        xbc_c = [p_xbc.tile([128, L], bf16, tag=f"xbc_c{t}",
                                            name=f"xbc_c{t}") for t in range(10)]
                        with tc.tile_pool(name="p_pre", bufs=1) as p_pre:
                            xbc_pre = [p_pre.tile([128, L + 3], bf16, tag=f"xbc_pre{t}",
                                                  name=f"xbc_pre{t}") for t in range(10)]

                            # ============ PHASE 1: in_proj ============
                            with (
                                tc.tile_pool(name="pA", bufs=1) as pA,
                                tc.tile_pool(name="ps1", bufs=4, space="PSUM") as ps1,
                                tc.tile_pool(name="pss", bufs=2, space="PSUM") as pss,
                            ):
                                xTr = xT_d.ap().rearrange("(ko p) t -> p ko t", p=128)
                                wir = w_in_d.ap().rearrange("(ko p) m -> p ko m", p=128)
                                xTs = pA.tile([128, 4, L], bf16, tag="xTs")
                                wis = pA.tile([128, 4, D_IN], bf16, tag="wis")
                                for k in range(4):
                                    nc.sync.dma_start(xTs[:, k, :], xTr[:, k, :])
                                    nc.sync.dma_start(wis[:, k, :], wir[:, k, :])

                                for t in range(10):
                                    nc.vector.memset(xbc_pre[t][:, 0:3], 0.0)

                                for m in [18] + list(range(8, 18)) + list(range(0, 8)):
                                    mp = 128 if m < 18 else 16
                                    for tb in range(4):
                                        tsl = slice(tb * 512, (tb + 1) * 512)
                                        ps = ps1.tile([128, 512], f32, tag="ps_inproj")
                                        for k in range(4):
                                            nc.tensor.matmul(
                                                ps[:mp, :],
                                                wis[:, k, m * 128: m * 128 + mp],
                                                xTs[:, k, tsl],
                                                start=(k == 0),
                                                stop=(k == 3),
                                            )
                                        if m < 8:
                                            nc.scalar.activation(sz[:, m, tsl], ps[:, :], AF.Silu)
                                        elif m < 18:
                                            t = m - 8
                                            nc.scalar.copy(
                                                xbc_pre[t][:, 3 + tb * 512: 3 + (tb + 1) * 512],
                                                ps[:, :])
                                        else:
                                            nc.scalar.copy(dtld[32:48, tsl], ps[:16, :])
                                    if m == 18 and tb == 3:
                                        # dt = softplus(pre) = ln(1 + exp(pre + dtb))
                                        nc.scalar.activation(dtld[32:48, :], dtld[32:48, :], AF.Exp,
                                                             bias=DTB[:, 0:1])
                                        nc.scalar.activation(dtld[0:16, :], dtld[32:48, :], AF.Ln,
                                                             bias=1.0)
                                        # logdA = -exp(A_log) * dt   (f32)
                                        nc.vector.tensor_scalar_mul(
                                            dtld[64:80, :], dtld[0:16, :], NAE[:, 0:1])

                                        # Atot per chunk = exp(chunk-sums of logdA)
                                        red = pss.tile([128, 32], f32, tag="small", name="red")
                                        nc.vector.tensor_reduce(
                                            red[0:16, 0:16],
                                            dtld[64:80, :].rearrange("p (c t) -> p c t", c=NCH),
                                            op=OP.add, axis=mybir.AxisListType.X,
                                        )
                                        nc.scalar.activation(atot[:], red[0:16, 0:16], AF.Exp)
                                        atT_ps = pss.tile([128, 32], f32, tag="small", name="atT_ps")
                                        nc.tensor.transpose(
                                            atT_ps[0:16, 0:16], atot[:], IDNF[0:16, 0:16])
                                        nc.vector.tensor_copy(atotT[:], atT_ps[0:16, 0:16])
                                        nc.sync.dma_start(
                                            atotF[:].rearrange("p (c h) -> p c h", c=16), atotT[:])

                                        # time-major dt/logdA per chunk via PE transpose
                                        for c in range(NCH):
                                            trp = pss.tile([128, 96], f32, tag="small2", name="trp")
                                            nc.tensor.transpose(
                                                trp[:], dtld[:, c * T:(c + 1) * T], IDNF[0:96, 0:96])
                                            nc.vector.tensor_copy(dtldT[:, c, :], trp[:])

                        # ============ PHASE 2-4 ============

                            # ---- conv (channel-major) ----
                            with tc.tile_pool(name="pC", bufs=2) as pC:
                                for t in [8, 9] + list(range(8)):
                                    acc = pC.tile([128, L], bf16, tag="conv_acc")
                                    nc.vector.tensor_scalar_mul(
                                        acc[:], xbc_pre[t][:, 0:L], CONVW[:, t, 0:1])
                                    for k in range(1, 4):
                                        nc.vector.scalar_tensor_tensor(
                                            acc[:], xbc_pre[t][:, k:k + L],
                                            CONVW[:, t, k:k + 1], acc[:],
                                            op0=OP.mult, op1=OP.add,
                                        )
                                    nc.scalar.activation(
                                        xbc_c[t][:], acc[:], AF.Silu,
                                        bias=CONVB[:, t:t + 1])
                                    nc.sync.dma_start(
                                        rt_dram[t * 128:(t + 1) * 128, :], xbc_c[t][:])

                        # ---- chunked scan ----
                        with (
                            tc.tile_pool(name="pS", bufs=3) as pS,
                            tc.tile_pool(name="psY", bufs=1, space="PSUM") as psY,
                            tc.tile_pool(name="psS", bufs=1, space="PSUM") as psS,
                            tc.tile_pool(name="psP", bufs=2, space="PSUM") as psP,
                            tc.tile_pool(name="psP1", bufs=1, space="PSUM") as psP1,
                            tc.tile_pool(name="psG", bufs=1, space="PSUM") as psG,
                        ):
                            # decay prep for all chunks up front
                            wdin_all = pS.tile([128, NCH, 32], f32, tag="wdin_all",
                                               name="wdin_all")
                            dtw_all = pS.tile([128, NCH, 16], f32, tag="dtw_all",
                                              name="dtw_all")
                            atb_all = pS.tile([128, NCH, 16], f32, tag="atb_all",
                                              name="atb_all")
                            for c in range(NCH):
                                ld_bf = pS.tile([128, 16], bf16, tag="ld_bf")
                                nc.vector.tensor_copy(ld_bf[:], dtldT[:, c, 64:80])
                                wdgt_ps = psG.tile([128, 176], f32, tag="wdgt_ps",
                                                   name="wdgt_ps")
                                wd_ps = wdgt_ps[:, 0:48]
                                nc.tensor.matmul(wd_ps[:, 0:16], ALOW[:], ld_bf[:],
                                                 start=True, stop=True)
                                nc.tensor.matmul(wd_ps[:, 16:32], UINC[:], ld_bf[:],
                                                 start=True, stop=True)
                                nc.scalar.activation(wdin_all[:, c, :], wd_ps[:, 0:32],
                                                     AF.Exp)
                                nc.vector.tensor_tensor(
                                    dtw_all[:, c, :], dtldT[:, c, 0:16],
                                    wdin_all[:, c, 0:16], op=OP.mult)
                                if c > 0:
                                    nc.tensor.matmul(
                                        wd_ps[:, 32:48], ONESRF[:],
                                        atotF[0:1, c * 16:(c + 1) * 16],
                                        start=True, stop=True)
                                    nc.vector.tensor_copy(atb_all[:, c, :],
                                                          wd_ps[:, 32:48])

                            for c in range(NCH):
                                csl = slice(c * T, (c + 1) * T)
                                wdin = wdin_all[:, c, :]
                                dtw = dtw_all[:, c, :]
                                at_bc = atb_all[:, c, :]
                                xbt = pS.tile([128, D_XBC], bf16, tag="xbt")
                                nc.sync.dma_start_transpose(xbt[:, 1024:1152],
                                                            rt_dram[1024:1152, csl])
                                nc.sync.dma_start_transpose(xbt[:, 1152:1280],
                                                            rt_dram[1152:1280, csl])
                                nc.sync.dma_start_transpose(xbt[:, 0:1024],
                                                            rt_dram[0:1024, csl])

                                # Gt = B @ C^T (shared across heads)
                                gt_ps = psG.tile([128, 176], f32, tag="wdgt_ps",
                                                 name="gt_wd")[:, 48:176]
                                nc.tensor.matmul(gt_ps[:], xbc_c[8][:, csl],
                                                 xbc_c[9][:, csl], start=True, stop=True)
                                gt = pS.tile([128, 128], bf16, tag="gt")
                                nc.scalar.copy(gt[:], gt_ps[:])

                                y_ps = psY.tile([128, 8, T], f32, tag="y_ps")
                                s_ps = psS.tile([128, NH, HD], f32, tag="s_ps")

                                for hb in range(4):
                                    e_ps = psP.tile([128, 4, 128], f32, tag="e_ps")
                                    cd_ps = psP1.tile([128, 4, 128], f32, tag="cd_ps")
                                    utmp4 = pS.tile([128, 4, 128], bf16, tag="utmp4")
                                    ddiag4 = pS.tile([128, 4, 128], bf16, tag="ddiag4")
                                    for hq in range(4):
                                        h = hb * 4 + hq
                                        ld_col = dtldT[:, c, 64 + h: 65 + h]
                                        nc.gpsimd.tensor_tensor(
                                            utmp4[:, hq, :], UINC[:],
                                            ld_col.to_broadcast([128, 128]), op=OP.mult)
                                        if c > 0:
                                            nc.scalar.activation(
                                                ddiag4[:, hq, :], IDNB[:], AF.Copy,
                                                scale=wdin[:, 16 + h: 17 + h])
                                    nc.tensor.matmul(e_ps[:], ALOW[:], utmp4[:],
                                                     start=True, stop=False)
                                    nc.tensor.matmul(e_ps[:], IDNB[:], MINF4[:],
                                                     start=False, stop=True)
                                    if c > 0:
                                        nc.tensor.matmul(cd_ps[:], xbt[:, 1152:1280],
                                                         ddiag4[:], start=True, stop=True)
                                    e_sb = pS.tile([128, 4, 128], bf16, tag="e_sb")
                                    nc.scalar.activation(e_sb[:], e_ps[:], AF.Exp)
                                    cd_sb = pS.tile([128, 4, 128], bf16, tag="cd_sb")
                                    if c > 0:
                                        nc.scalar.copy(cd_sb[:], cd_ps[:])

                                    for hq in range(4):
                                        h = hb * 4 + hq
                                        m_sb = pS.tile([128, 128], bf16, tag="m_sb")
                                        nc.vector.scalar_tensor_tensor(
                                            m_sb[:], gt[:], dtldT[:, c, h: h + 1],
                                            e_sb[:, hq, :], op0=OP.mult, op1=OP.mult)
                                        xdw = pS.tile([128, HD], bf16, tag="xdw")
                                        nc.vector.tensor_scalar_mul(
                                            xdw[:], xbt[:, h * HD:(h + 1) * HD],
                                            dtw[:, h: h + 1])

                                        ph, fh = (h % 2) * 64, h // 2
                                        nc.tensor.matmul(
                                            y_ps[ph:ph + 64, fh, :],
                                            xbt[:, h * HD:(h + 1) * HD], m_sb[:],
                                            start=True, stop=(c == 0))
                                        if c > 0:
                                            nc.tensor.matmul(
                                                y_ps[ph:ph + 64, fh, :],
                                                s_sb[(c + 1) % 2][:, h, :],
                                                cd_sb[:, hq, :],
                                                start=False, stop=True,
                                                skip_group_check=True)
                                        nc.tensor.matmul(
                                            s_ps[:, h, :], xbt[:, 1024:1152], xdw[:],
                                            start=True, stop=True)


                                for t in range(8):
                                    nc.vector.scalar_tensor_tensor(
                                        y_sb[:, t, csl], xbc_c[t][:, csl],
                                        DCOL[:, t:t + 1], y_ps[:, t, :],
                                        op0=OP.mult, op1=OP.add)
                                # state evac: S_new = S_old * atot + S_psum
                                if c == 0:
                                    nc.vector.tensor_copy(s_sb[0][:], s_ps[:])
                                else:
                                    s_scaled = pS.tile([128, NH, HD], bf16, tag="s_scaled")
                                    nc.gpsimd.tensor_tensor(
                                        s_scaled[:], s_sb[(c + 1) % 2][:],
                                        at_bc[:, :, None].to_broadcast([128, NH, HD]),
                                        op=OP.mult)
                                    nc.vector.tensor_tensor(
                                        s_sb[c % 2][:], s_scaled[:], s_ps[:], op=OP.add)

                    # ============ PHASE 5: gating + RMSNorm ============
                    with tc.tile_pool(name="p_late", bufs=1) as p_late:
                        g = p_late.tile([128, 8, L], bf16, tag="g")
                        gn = p_late.tile([128, 8, L], bf16, tag="gn")
                        rstd_cols = p_late.tile([128, 16], f32, tag="rstd_cols")
                        rstdT = p_late.tile([16, 128], bf16, tag="rstdT")
                        rstdF = p_late.tile([1, NCH * 128], bf16, tag="rstdF")
                        rstd_bc = p_late.tile([128, L], bf16, tag="rstd_bc")
                        with (
                            tc.tile_pool(name="pG", bufs=3) as pG,
                            tc.tile_pool(name="psN", bufs=2, space="PSUM") as psN,
                        ):
                            for c in range(NCH):
                                csl = slice(c * T, (c + 1) * T)
                                nc.vector.tensor_tensor(
                                    g[:, :, csl], y_sb[:, :, csl], sz[:, :, csl], op=OP.mult)
                                g2 = pG.tile([128, 8, T], bf16, tag="g2")
                                nc.scalar.square(g2[:], g[:, :, csl])
                                ss_ps = psN.tile([128, 1], f32, tag="ss_ps")
                                for t in range(8):
                                    nc.tensor.matmul(ss_ps[:], g2[:, t, :], ONEC[:],
                                                     start=(t == 0), stop=(t == 7))
                                lnv = pG.tile([128, 1], f32, tag="lnv")
                                nc.scalar.activation(lnv[:], ss_ps[:], AF.Ln,
                                                     bias=EPSC[:, 0:1], scale=1.0 / D_INNER)
                                nc.scalar.activation(rstd_cols[:, c: c + 1], lnv[:],
                                                     AF.Exp, scale=-0.5)
                            rs_ps = psN.tile([16, 128], f32, tag="rs_ps")
                            nc.tensor.transpose(rs_ps[:], rstd_cols[:], IDNF[:])
                            nc.vector.tensor_copy(rstdT[:], rs_ps[:])
                            nc.sync.dma_start(
                                rstdF[:].rearrange("p (c t) -> p c t", c=NCH), rstdT[:])
                            for c in range(NCH):
                                rb_ps = psN.tile([128, 128], f32, tag="rb_ps")
                                nc.tensor.matmul(
                                    rb_ps[:], ONESRB[:],
                                    rstdF[0:1, c * T:(c + 1) * T],
                                    start=True, stop=True)
                                nc.vector.tensor_copy(rstd_bc[:, c * T:(c + 1) * T], rb_ps[:])
                            for t in range(8):
                                nc.vector.scalar_tensor_tensor(
                                    gn[:, t, :], g[:, t, :], NRMW[:, t:t + 1], rstd_bc[:],
                                    op0=OP.mult, op1=OP.mult)

                        # ============ PHASE 6: out_proj ============
                        with (
                            tc.tile_pool(name="pO", bufs=1) as pO,
                            tc.tile_pool(name="psO", bufs=4, space="PSUM") as psO,
                        ):
                            wo = pO.tile([128, 8, D_MODEL], bf16, tag="wo")
                            for k in range(8):
                                wof = pO.tile([128, D_MODEL], f32, tag="wof")
                                nc.sync.dma_start(
                                    wof[:], w_out_d.ap()[k * 128:(k + 1) * 128, :])
                                nc.scalar.copy(wo[:, k, :], wof[:])
                            yT_sb = pO.tile([128, 4, L], f32, tag="yT_sb")
                            for tb in range(4):
                                tsl = slice(tb * 512, (tb + 1) * 512)
                                for m in range(4):
                                    ps = psO.tile([128, 512], f32, tag="ps_out")
                                    for k in range(8):
                                        nc.tensor.matmul(
                                            ps[:], wo[:, k, m * 128:(m + 1) * 128],
                                            gn[:, k, tsl],
                                            start=(k == 0), stop=(k == 7))
                                    nc.scalar.copy(yT_sb[:, m, tsl], ps[:])
                            nc.sync.dma_start(
                                yT_d.ap().rearrange("(mo p) t -> p mo t", p=128), yT_sb[:])

    _fix_waits(nc, mybir)

    return nc


def _fix_waits(nc, mybir):
    """This walrus build supports one sem-wait slot per instruction; hoist
    excess waits onto preceding NoOps on the same engine."""
    nwn = [0]
    for bb in nc.main_func.blocks:
        newl = []
        changed = False
        for inst in bb.instructions:
            si = inst.sync_info
            waits = list(si.on_wait) if (si and si.on_wait) else []
            if len(waits) > 1:
                imm = [w for w in waits if w.wait_reg is None]
                reg = [w for w in waits if w.wait_reg is not None]
                keep = (reg + imm)[:1]
                spill = [w for w in waits if w not in keep]
                assert not any(w.wait_reg is not None for w in spill), inst.name
                for w in spill:
                    nwn[0] += 1
                    nop = mybir.InstNoOp(name=f"I-wsplit-{nwn[0]}", ins=[], outs=[])
                    nop.engine = inst.engine
                    nop.sync_info = mybir.SyncInfo(on_wait=[w], on_update=[])
                    nc.register_instruction(nop)
                    newl.append(nop)
                si.on_wait = keep
                changed = True
            newl.append(inst)
        if changed:
            bb.instructions = newl


def _get_program():
    if "nc" not in _CACHE:
        _CACHE["nc"] = _build_program()
    return _CACHE["nc"]


def _host_consts():
    if "consts" in _CACHE:
        return _CACHE["consts"]
    import ml_dtypes
    k = np.arange(128)
    alow = (k[:, None] > k[None, :]).astype(np.float32)      # [k > j]
    uinc = (k[:, None] <= k[None, :]).astype(np.float32)     # [k <= i]
    idn = np.eye(128, dtype=np.float32)
    consts = dict(
        alow=alow.astype(ml_dtypes.bfloat16),
        uinc=uinc.astype(ml_dtypes.bfloat16),
        idnb=idn.astype(ml_dtypes.bfloat16),
        idnf=idn,
        ones=np.ones((128, 1), ml_dtypes.bfloat16),
        onesrf=np.ones((1, 128), np.float32),
        onesrb=np.ones((1, 128), ml_dtypes.bfloat16),
        minf4=np.tile((k[:, None] > k[None, :]).astype(np.float32) * NEG_INF,
                      (1, 4)).astype(ml_dtypes.bfloat16),
    )
    _CACHE["consts"] = consts
    return consts


def _core_inputs(x_seq, p):
    """x_seq: (L, D_MODEL) f32 (already flipped for bw); p: dict of params."""
    import ml_dtypes
    consts = _host_consts()
    dcol = p["D"].astype(np.float32).repeat(HD).reshape(8, 128).T.copy()
    nrmw = p["norm_w"].astype(np.float32).reshape(8, 128).T.copy()
    convw = np.ascontiguousarray(
        p["conv_w"].astype(np.float32).reshape(4, 10, 128).transpose(2, 1, 0)
    )
    convb = np.ascontiguousarray(p["conv_b"].astype(np.float32).reshape(10, 128).T)
    return dict(
        xT=np.ascontiguousarray(x_seq.T).astype(ml_dtypes.bfloat16),
        w_in=np.ascontiguousarray(p["in_proj"]).astype(ml_dtypes.bfloat16),
        w_out=np.ascontiguousarray(p["out_proj"].astype(np.float32)),
        convw=convw,
        convb=convb,
        dtb=p["dt_bias"].astype(np.float32).reshape(16, 1),
        nae=(-np.exp(p["A_log"].astype(np.float32))).reshape(16, 1),
        dcol=dcol,
        nrmw=nrmw,
        **consts,
    )


def kernel(**inputs):
    from concourse.bass_utils import run_bass_kernel_spmd

    nc = _get_program()
    x = np.asarray(inputs["x"], np.float32)
    mask = np.asarray(inputs["padding_mask"])

    def params(pre):
        names = ["in_proj", "conv_w", "conv_b", "dt_bias", "A_log", "D", "norm_w", "out_proj"]
        return {n: np.asarray(inputs[pre + n]) for n in names}

    pf, pb = params("fw_"), params("bw_")
    in_maps = []
    for b in range(B_SZ):
        in_maps.append(_core_inputs(x[b], pf))
    for b in range(B_SZ):
        in_maps.append(_core_inputs(x[b][::-1], pb))

    res = run_bass_kernel_spmd(nc, in_maps, core_ids=list(range(8)))
    out = np.empty((B_SZ, L, D_MODEL), np.float32)
    for b in range(B_SZ):
        yf = res.results[b]["yT"].T
        yb = res.results[B_SZ + b]["yT"].T[::-1]
        out[b] = yf + yb
    out[mask] = 0.0
    return out



# revision 5
# speedup vs baseline: 1.0021x; 1.0021x over previous
"""Bidirectional Mamba2 layer on 8 NeuronCores.

Sharding: 8 cores = 4 batch elements x 2 directions (fw/bw). Each core runs
one full Mamba2 layer pass on one sequence; the host flips the bw sequences,
adds fw+bw results, and applies the padding mask.

Per-core kernel (sequence length L=2048, chunked SSD scan with T=128),
pipelined per 512-token block (tb):
  1. dt block of in_proj first (softplus, log-decay, per-chunk decay prep).
  2. per tb: xBC in_proj matmuls -> causal conv (DVE+Pool taps, Silu on ACT)
     -> DRAM roundtrip with DMA-transpose -> 4 scan chunks.
     Scan uses 4-head-batched ops: xdt = x*dt (DVE), xdw = xdt*decay (Pool),
     segsum via ALOW matmul on utmp4 (Pool), m4 = (gt*tril) * exp(segsum)
     (mask folded into gt, dt folded into xdt), state matmul batched over
     8 heads, D*x folded into conv output in-place.
  3. per tb: z in_proj + silu -> gating (in-place) -> RMSNorm (norm_w folded
     into out_proj on host) -> out_proj -> DMA out.
"""

import numpy as np

D_MODEL = 512
D_STATE = 128
NH = 16
HD = 64
D_INNER = 1024
D_XBC = 1280
D_IN = 2320
L = 2048
T = 128
NCH = L // T
B_SZ = 4
EPS = 1e-5

_CACHE = {}


def _patch_drain(tile, mybir, ScopedClock):
    # workaround: this walrus build rejects >2 sem waits per instruction;
    # spread the TileContext exit-drain waits across nop instructions.
    def _drain_and_barrier(self, tick_clock, wait_clock):
        nc_ = self.nc
        probe = nc_.sync.nop()
        wait_clock.add_sem_waits(
            probe.ins, ScopedClock({None: tick_clock.global_clock})
        )
        waits = list(probe.ins.sync_info.on_wait or [])
        if probe.ins.sync_info is not None:
            probe.ins.sync_info.on_wait = waits[:1]
            rest = waits[1:]
        else:
            rest = []
        for w in rest:
            n = nc_.sync.nop()
            if n.ins.sync_info is None:
                n.ins.sync_info = mybir.SyncInfo(on_wait=[w], on_update=[])
            else:
                n.ins.sync_info.on_wait = [w]
        nc_.sync.drain()
        nc_.all_engine_barrier()
        assert self.sems is not None
        popped = nc_._tile_sem_poison_stack.pop()
        assert popped is self._sem_poison
        nc_.clear_and_free_semaphores(list(self.sems.allocated().values()))
        nc_.all_engine_barrier()

    tile.TileContext._drain_and_barrier = _drain_and_barrier


def _build_program():
    import concourse.bass as bass
    import concourse.mybir as mybir
    import concourse.tile as tile
    from concourse.vector_clock import ScopedClock

    _patch_drain(tile, mybir, ScopedClock)

    f32 = mybir.dt.float32
    bf16 = mybir.dt.bfloat16
    AF = mybir.ActivationFunctionType
    OP = mybir.AluOpType

    nc = bass.Bass("TRN2", target_bir_lowering=False, debug=False)

    # ---------------- DRAM I/O ----------------
    xT_d = nc.dram_tensor("xT", [D_MODEL, L], bf16, kind="ExternalInput")
    w_in_d = nc.dram_tensor("w_in", [D_MODEL, D_IN], bf16, kind="ExternalInput")
    w_out_d = nc.dram_tensor("w_out", [D_INNER, D_MODEL], bf16, kind="ExternalInput")
    convw_d = nc.dram_tensor("convw", [128, 10, 4], f32, kind="ExternalInput")
    convb_d = nc.dram_tensor("convb", [128, 10], f32, kind="ExternalInput")
    dtb_d = nc.dram_tensor("dtb", [16, 1], f32, kind="ExternalInput")
    nae_d = nc.dram_tensor("nae", [16, 1], f32, kind="ExternalInput")  # -exp(A_log)
    dcol_d = nc.dram_tensor("dcol", [128, 8], f32, kind="ExternalInput")  # D per pair-tile
    alow_d = nc.dram_tensor("alow", [128, 128], bf16, kind="ExternalInput")
    uinc_d = nc.dram_tensor("uinc", [128, 128], bf16, kind="ExternalInput")
    idnb_d = nc.dram_tensor("idnb", [128, 128], bf16, kind="ExternalInput")
    idnf_d = nc.dram_tensor("idnf", [128, 128], f32, kind="ExternalInput")
    ones_d = nc.dram_tensor("ones", [128, 1], bf16, kind="ExternalInput")
    onesrf_d = nc.dram_tensor("onesrf", [1, 128], f32, kind="ExternalInput")
    onesrb_d = nc.dram_tensor("onesrb", [1, 128], bf16, kind="ExternalInput")
    yT_d = nc.dram_tensor("yT", [D_MODEL, L], f32, kind="ExternalOutput")

    with tile.TileContext(nc) as tc:
        with (
            tc.tile_pool(name="const", bufs=1) as cpool,
            tc.tile_pool(name="dram", bufs=1, space="DRAM") as dpool,
            tc.tile_pool(name="mid", bufs=1) as mid,
            tc.tile_pool(name="pA", bufs=1) as pA,
        ):
            # ---------------- constants ----------------
            ALOW = cpool.tile([128, 128], bf16, tag="alow")
            nc.sync.dma_start(ALOW[:], alow_d.ap())
            UINC = cpool.tile([128, 128], bf16, tag="uinc")
            nc.sync.dma_start(UINC[:], uinc_d.ap())
            IDNB = cpool.tile([128, 128], bf16, tag="idnb")
            nc.sync.dma_start(IDNB[:], idnb_d.ap())
            IDNF = cpool.tile([128, 128], f32, tag="idnf")
            nc.sync.dma_start(IDNF[:], idnf_d.ap())
            ONEC = cpool.tile([128, 1], bf16, tag="ones")
            nc.sync.dma_start(ONEC[:], ones_d.ap())
            ONESRF = cpool.tile([1, 128], f32, tag="onesrf")
            nc.sync.dma_start(ONESRF[:], onesrf_d.ap())
            ONESRB = cpool.tile([1, 128], bf16, tag="onesrb")
            nc.sync.dma_start(ONESRB[:], onesrb_d.ap())
            CONVW = cpool.tile([128, 10, 4], f32, tag="convw")
            nc.sync.dma_start(CONVW[:], convw_d.ap())
            CONVB = cpool.tile([128, 10], f32, tag="convb")
            nc.sync.dma_start(CONVB[:], convb_d.ap())
            DTB = cpool.tile([16, 1], f32, tag="dtb")
            nc.sync.dma_start(DTB[:], dtb_d.ap())
            NAE = cpool.tile([16, 1], f32, tag="nae")
            nc.sync.dma_start(NAE[:], nae_d.ap())
            DCOL = cpool.tile([128, 8], f32, tag="dcol")
            nc.sync.dma_start(DCOL[:], dcol_d.ap())
            EPSC = cpool.tile([128, 1], f32, tag="epsc")
            nc.vector.memset(EPSC[:], EPS)

            # ---------------- small persistent tensors ----------------
            dtld = mid.tile([96, L], f32, tag="dtld")           # dt rows 0:16, logdA 64:80
            dtldT = mid.tile([128, NCH, 96], f32, tag="dtldT")  # time-major dt/logdA
            atot = mid.tile([16, 16], f32, tag="atot")          # [head, chunk]
            atotT = mid.tile([16, 16], f32, tag="atotT")        # [chunk, head]
            s_sb = [mid.tile([128, NH, HD], bf16, tag=f"s_sb{i}", name=f"s_sb{i}")
                    for i in range(2)]
            atotF = mid.tile([1, 256], f32, tag="atotF")
            wdin_all = mid.tile([128, NCH, 32], f32, tag="wdin_all")
            atb_all = mid.tile([128, NCH, 16], f32, tag="atb_all")

            rt_dram = dpool.tile([D_XBC, L], bf16)              # roundtrip buffer

            with tc.tile_pool(name="p_ysb", bufs=1) as p_ysb:
                y_sb = p_ysb.tile([128, 8, L], bf16, tag="y_sb")
                with tc.tile_pool(name="p_xbc", bufs=1) as p_xbc:
                    xbc_x = p_xbc.tile([128, 8, L], bf16, tag="xbc_x")
                    xbc_B = p_xbc.tile([128, L], bf16, tag="xbc_B")
                    xbc_C = p_xbc.tile([128, L], bf16, tag="xbc_C")

                    with (
                        tc.tile_pool(name="p_pre", bufs=1) as p_pre,
                        tc.tile_pool(name="pS", bufs=2) as pS,
                        tc.tile_pool(name="pXB", bufs=2) as pXB,
                        tc.tile_pool(name="pC", bufs=2) as pC,
                        tc.tile_pool(name="psIn", bufs=2, space="PSUM") as psIn,
                        tc.tile_pool(name="psY", bufs=1, space="PSUM") as psY,
                        tc.tile_pool(name="psS", bufs=1, space="PSUM") as psS,
                        tc.tile_pool(name="psE", bufs=2, space="PSUM") as psE,
                    ):
                        xbc_pre = p_pre.tile([128, 10, L + 3], bf16, tag="xbc_pre")
                        nc.vector.memset(xbc_pre[:, :, 0:3], 0.0)

                        xTr = xT_d.ap().rearrange("(ko p) t -> p ko t", p=128)
                        wir = w_in_d.ap().rearrange("(ko p) m -> p ko m", p=128)
                        xTs = pA.tile([128, 4, L], bf16, tag="xTs")
                        wis = pA.tile([128, 4, D_IN], bf16, tag="wis")
                        for k in range(4):
                            nc.sync.dma_start(xTs[:, k, :], xTr[:, k, :])
                            nc.sync.dma_start(wis[:, k, :], wir[:, k, :])

                        # ---- dt block of in_proj (m = 18), all tb ----
                        for tb in range(4):
                            tsl = slice(tb * 512, (tb + 1) * 512)
                            ps = psIn.tile([128, 512], f32, tag="ps_in")
                            for k in range(4):
                                nc.tensor.matmul(
                                    ps[:16, :], wis[:, k, 18 * 128: 18 * 128 + 16],
                                    xTs[:, k, tsl], start=(k == 0), stop=(k == 3))
                            nc.scalar.copy(dtld[32:48, tsl], ps[:16, :])
                        # dt = softplus(pre) = ln(1 + exp(pre + dtb))
                        nc.scalar.activation(dtld[32:48, :], dtld[32:48, :], AF.Exp,
                                             bias=DTB[:, 0:1])
                        nc.scalar.activation(dtld[0:16, :], dtld[32:48, :], AF.Ln,
                                             bias=1.0)
                        # logdA = -exp(A_log) * dt   (f32)
                        nc.vector.tensor_scalar_mul(
                            dtld[64:80, :], dtld[0:16, :], NAE[:, 0:1])

                        # Atot per chunk = exp(chunk-sums of logdA)
                        red = psIn.tile([128, 512], f32, tag="ps_in", name="red")
                        nc.vector.tensor_reduce(
                            red[0:16, 0:16],
                            dtld[64:80, :].rearrange("p (c t) -> p c t", c=NCH),
                            op=OP.add, axis=mybir.AxisListType.X,
                        )
                        nc.scalar.activation(atot[:], red[0:16, 0:16], AF.Exp)
                        atT_ps = psIn.tile([128, 512], f32, tag="ps_in", name="atT_ps")
                        nc.tensor.transpose(
                            atT_ps[0:16, 0:16], atot[:], IDNF[0:16, 0:16])
                        nc.vector.tensor_copy(atotT[:], atT_ps[0:16, 0:16])
                        nc.sync.dma_start(
                            atotF[:].rearrange("p (c h) -> p c h", c=16), atotT[:])

                        # time-major dt/logdA per chunk via PE transpose
                        for c in range(NCH):
                            trp = psIn.tile([128, 512], f32, tag="ps_in", name="trp")
                            nc.tensor.transpose(
                                trp[:, 0:96], dtld[:, c * T:(c + 1) * T],
                                IDNF[0:96, 0:96])
                            nc.vector.tensor_copy(dtldT[:, c, :], trp[:, 0:96])

                        # ---- per-chunk decay prep ----
                        for c in range(NCH):
                            ld_bf = pS.tile([128, 16], bf16, tag="ld_bf")
                            nc.vector.tensor_copy(ld_bf[:], dtldT[:, c, 64:80])
                            wd_ps = psE.tile([128, 4, 128], f32, tag="ps_e",
                                             name="wd_ps")
                            nc.tensor.matmul(wd_ps[:, 0, 0:16], ALOW[:], ld_bf[:],
                                             start=True, stop=True)
                            nc.tensor.matmul(wd_ps[:, 0, 16:32], UINC[:], ld_bf[:],
                                             start=True, stop=True)
                            nc.scalar.activation(wdin_all[:, c, :],
                                                 wd_ps[:, 0, 0:32], AF.Exp)
                            if c > 0:
                                nc.tensor.matmul(
                                    wd_ps[:, 0, 32:48], ONESRF[:],
                                    atotF[0:1, c * 16:(c + 1) * 16],
                                    start=True, stop=True)
                                nc.vector.tensor_copy(atb_all[:, c, :],
                                                      wd_ps[:, 0, 32:48])

                        # ======== pipelined tb loop: in_proj xBC / conv / scan ====
                        for tb in range(4):
                            tsl = slice(tb * 512, (tb + 1) * 512)
                            # ---- in_proj xBC blocks (B, C first, then x) ----
                            for t in [8, 9] + list(range(8)):
                                m = 8 + t
                                ps = psIn.tile([128, 512], f32, tag="ps_in")
                                for k in range(4):
                                    nc.tensor.matmul(
                                        ps[:], wis[:, k, m * 128:(m + 1) * 128],
                                        xTs[:, k, tsl],
                                        start=(k == 0), stop=(k == 3))
                                nc.scalar.copy(
                                    xbc_pre[:, t, 3 + tb * 512: 3 + (tb + 1) * 512],
                                    ps[:])

                            # ---- conv (channel-major) for this tb ----
                            for t in [8, 9] + list(range(8)):
                                acc = pC.tile([128, 512], bf16, tag="conv_acc")
                                base = tb * 512
                                nc.scalar.activation(
                                    acc[:], xbc_pre[:, t, base:base + 512],
                                    AF.Copy, scale=CONVW[:, t, 0:1])
                                for k in (1, 2, 3):
                                    nc.vector.scalar_tensor_tensor(
                                        acc[:], xbc_pre[:, t, base + k:base + k + 512],
                                        CONVW[:, t, k:k + 1], acc[:],
                                        op0=OP.mult, op1=OP.add)
                                if t < 8:
                                    dest = xbc_x[:, t, tsl]
                                elif t == 8:
                                    dest = xbc_B[:, tsl]
                                else:
                                    dest = xbc_C[:, tsl]
                                nc.scalar.activation(dest, acc[:], AF.Silu,
                                                     bias=CONVB[:, t:t + 1])
                                nc.sync.dma_start(
                                    rt_dram[t * 128:(t + 1) * 128, tsl], dest)

                            # ---- fold D into x in-place (y eviction uses it) ----
                            nc.gpsimd.tensor_tensor(
                                xbc_x[:, :, tsl], xbc_x[:, :, tsl],
                                DCOL[:, :, None].to_broadcast([128, 8, 512]),
                                op=OP.mult)

                            # ---- scan: 4 chunks of this tb ----
                            for ci in range(4):
                                c = 4 * tb + ci
                                csl = slice(c * T, (c + 1) * T)
                                wdin = wdin_all[:, c, :]

                                xbt = pXB.tile([128, D_XBC], bf16, tag="xbt")
                                nc.sync.dma_start_transpose(xbt[:, 1024:1152],
                                                            rt_dram[1024:1152, csl])
                                nc.sync.dma_start_transpose(xbt[:, 1152:1280],
                                                            rt_dram[1152:1280, csl])
                                nc.sync.dma_start_transpose(xbt[:, 0:1024],
                                                            rt_dram[0:1024, csl])

                                # Gt = B @ C^T (shared across heads), tril mask
                                gt_ps = psE.tile([128, 4, 128], f32, tag="ps_e",
                                                 name="gt_ps")
                                nc.tensor.matmul(gt_ps[:, 0, :], xbc_B[:, csl],
                                                 xbc_C[:, csl], start=True, stop=True)
                                gt = pS.tile([128, 128], bf16, tag="gt")
                                nc.vector.tensor_tensor(gt[:], gt_ps[:, 0, :],
                                                        UINC[:], op=OP.mult)

                                # xdt = x * dt, xdw = xdt * decay (16 heads at once)
                                xdt = pS.tile([128, NH, HD], bf16, tag="xdt")
                                nc.vector.tensor_tensor(
                                    xdt[:],
                                    xbt[:, 0:1024].rearrange("p (h d) -> p h d", h=16),
                                    dtldT[:, c, 0:16][:, :, None]
                                    .to_broadcast([128, NH, HD]),
                                    op=OP.mult)
                                xdw = pS.tile([128, NH, HD], bf16, tag="xdw")
                                nc.gpsimd.tensor_tensor(
                                    xdw[:], xdt[:],
                                    wdin[:, 0:16][:, :, None]
                                    .to_broadcast([128, NH, HD]),
                                    op=OP.mult)

                                y_ps = psY.tile([128, 8, T], f32, tag="y_ps")
                                s_ps = psS.tile([128, NH, HD], f32, tag="s_ps")

                                for hb in range(4):
                                    hsl = slice(64 + 4 * hb, 68 + 4 * hb)
                                    utmp4 = pS.tile([128, 4, 128], bf16, tag="utmp4")
                                    nc.gpsimd.tensor_tensor(
                                        utmp4[:],
                                        UINC[:, None, :].to_broadcast([128, 4, 128]),
                                        dtldT[:, c, hsl][:, :, None]
                                        .to_broadcast([128, 4, 128]),
                                        op=OP.mult)
                                    e_ps = psE.tile([128, 4, 128], f32, tag="ps_e",
                                                    name="e_ps")
                                    nc.tensor.matmul(e_ps[:], ALOW[:], utmp4[:],
                                                     start=True, stop=True)
                                    e4 = pS.tile([128, 4, 128], bf16, tag="e4")
                                    nc.scalar.activation(e4[:], e_ps[:], AF.Exp)
                                    m4 = pS.tile([128, 4, 128], bf16, tag="m4")
                                    nc.vector.tensor_tensor(
                                        m4[:],
                                        gt[:, None, :].to_broadcast([128, 4, 128]),
                                        e4[:], op=OP.mult)
                                    if c > 0:
                                        ddiag4 = pS.tile([128, 4, 128], bf16,
                                                         tag="ddiag4")
                                        nc.gpsimd.tensor_tensor(
                                            ddiag4[:],
                                            IDNB[:, None, :]
                                            .to_broadcast([128, 4, 128]),
                                            wdin[:, 16 + 4 * hb: 20 + 4 * hb]
                                            [:, :, None].to_broadcast([128, 4, 128]),
                                            op=OP.mult)
                                        cd_ps = psE.tile([128, 4, 128], f32,
                                                         tag="ps_e", name="cd_ps")
                                        nc.tensor.matmul(cd_ps[:], xbt[:, 1152:1280],
                                                         ddiag4[:], start=True,
                                                         stop=True)
                                        cd_sb = pS.tile([128, 4, 128], bf16,
                                                        tag="cd_sb")
                                        nc.scalar.copy(cd_sb[:], cd_ps[:])

                                    for hq in range(4):
                                        h = hb * 4 + hq
                                        ph, fh = (h % 2) * 64, h // 2
                                        nc.tensor.matmul(
                                            y_ps[ph:ph + 64, fh, :],
                                            xdt[:, h, :], m4[:, hq, :],
                                            start=True, stop=(c == 0))
                                        if c > 0:
                                            nc.tensor.matmul(
                                                y_ps[ph:ph + 64, fh, :],
                                                s_sb[(c + 1) % 2][:, h, :],
                                                cd_sb[:, hq, :],
                                                start=False, stop=True,
                                                skip_group_check=True)

                                # state: S_chunk = B^T @ (x*dt*w), 8 heads per matmul
                                nc.tensor.matmul(s_ps[:, 0:8, :], xbt[:, 1024:1152],
                                                 xdw[:, 0:8, :], start=True, stop=True)
                                nc.tensor.matmul(s_ps[:, 8:16, :], xbt[:, 1024:1152],
                                                 xdw[:, 8:16, :], start=True,
                                                 stop=True)

                                # y eviction: y = D*x + y_psum (D pre-folded into x)
                                nc.vector.tensor_tensor(
                                    y_sb[:, :, csl], xbc_x[:, :, csl], y_ps[:],
                                    op=OP.add)

                                # state evac: S_new = S_old * atot + S_psum
                                if c == 0:
                                    nc.vector.tensor_copy(s_sb[0][:], s_ps[:])
                                else:
                                    s_scaled = pS.tile([128, NH, HD], bf16,
                                                       tag="s_scaled")
                                    nc.gpsimd.tensor_tensor(
                                        s_scaled[:], s_sb[(c + 1) % 2][:],
                                        atb_all[:, c, :][:, :, None]
                                        .to_broadcast([128, NH, HD]),
                                        op=OP.mult)
                                    nc.vector.tensor_tensor(
                                        s_sb[c % 2][:], s_scaled[:], s_ps[:],
                                        op=OP.add)

                # ======== z in_proj + gating + RMSNorm + out_proj ========
                with (
                    tc.tile_pool(name="p_late", bufs=1) as p_late,
                    tc.tile_pool(name="pG", bufs=2) as pG,
                    tc.tile_pool(name="pO", bufs=2) as pO,
                    tc.tile_pool(name="psO", bufs=2, space="PSUM") as psO,
                    tc.tile_pool(name="psN", bufs=2, space="PSUM") as psN,
                ):
                    sz = p_late.tile([128, 8, L], bf16, tag="sz")
                    wo = p_late.tile([128, 8, D_MODEL], bf16, tag="wo")
                    rstd_cols = p_late.tile([128, 16], f32, tag="rstd_cols")
                    rstdT = p_late.tile([16, 128], bf16, tag="rstdT")
                    rstdF = p_late.tile([1, L], bf16, tag="rstdF")
                    wor = w_out_d.ap().rearrange("(ko p) m -> p ko m", p=128)
                    nc.sync.dma_start(wo[:], wor)

                    for tb in range(4):
                        tsl = slice(tb * 512, (tb + 1) * 512)
                        # ---- z blocks of in_proj -> silu -> sz ----
                        for m in range(8):
                            ps = psO.tile([128, 512], f32, tag="ps_o")
                            for k in range(4):
                                nc.tensor.matmul(
                                    ps[:], wis[:, k, m * 128:(m + 1) * 128],
                                    xTs[:, k, tsl], start=(k == 0), stop=(k == 3))
                            nc.scalar.activation(sz[:, m, tsl], ps[:], AF.Silu)
                        # ---- gating: g = y * silu(z), in place into sz ----
                        nc.vector.tensor_tensor(sz[:, :, tsl], sz[:, :, tsl],
                                                y_sb[:, :, tsl], op=OP.mult)
                        # ---- RMSNorm factors per chunk ----
                        for ci in range(4):
                            c = 4 * tb + ci
                            csl = slice(c * T, (c + 1) * T)
                            g2 = pG.tile([128, 8, T], bf16, tag="g2")
                            nc.vector.tensor_tensor(g2[:], sz[:, :, csl],
                                                    sz[:, :, csl], op=OP.mult)
                            ssn = psN.tile([128, 128], f32, tag="ps_n")
                            for t in range(8):
                                nc.tensor.matmul(ssn[:, 0:1], g2[:, t, :], ONEC[:],
                                                 start=(t == 0), stop=(t == 7))
                            lnv = pG.tile([128, 1], f32, tag="lnv")
                            nc.scalar.activation(lnv[:], ssn[:, 0:1], AF.Ln,
                                                 bias=EPSC[:, 0:1],
                                                 scale=1.0 / D_INNER)
                            nc.scalar.activation(rstd_cols[:, c:c + 1], lnv[:],
                                                 AF.Exp, scale=-0.5)
                        # transpose + broadcast rstd over channels
                        rsn = psN.tile([128, 128], f32, tag="ps_n", name="rsn")
                        nc.tensor.transpose(rsn[0:4, 0:128],
                                            rstd_cols[:, 4 * tb:4 * tb + 4], IDNF[:])
                        nc.vector.tensor_copy(rstdT[0:4, :], rsn[0:4, 0:128])
                        nc.sync.dma_start(
                            rstdF[0:1, tsl].rearrange("p (c t) -> p c t", c=4),
                            rstdT[0:4, :])
                        rstd_bc = pG.tile([128, 512], bf16, tag="rstd_bc")
                        for ci in range(4):
                            c = 4 * tb + ci
                            rbn = psN.tile([128, 128], f32, tag="ps_n", name="rbn")
                            nc.tensor.matmul(rbn[:], ONESRB[:],
                                             rstdF[0:1, c * T:(c + 1) * T],
                                             start=True, stop=True)
                            nc.vector.tensor_copy(rstd_bc[:, ci * T:(ci + 1) * T],
                                                  rbn[:])
                        # gn = g * rstd (norm_w folded into w_out on host)
                        nc.vector.tensor_tensor(
                            sz[:, :, tsl], sz[:, :, tsl],
                            rstd_bc[:, None, :].to_broadcast([128, 8, 512]),
                            op=OP.mult)
                        # ---- out_proj ----
                        for mo in range(4):
                            ps = psO.tile([128, 512], f32, tag="ps_o")
                            for k in range(8):
                                nc.tensor.matmul(
                                    ps[:], wo[:, k, mo * 128:(mo + 1) * 128],
                                    sz[:, k, tsl], start=(k == 0), stop=(k == 7))
                            yTs = pO.tile([128, 512], f32, tag="yTs")
                            nc.scalar.copy(yTs[:], ps[:])
                            nc.sync.dma_start(
                                yT_d.ap()[mo * 128:(mo + 1) * 128, tsl], yTs[:])

    _fix_waits(nc, mybir)

    return nc


def _fix_waits(nc, mybir):
    """This walrus build supports one sem-wait slot per instruction; hoist
    excess waits onto preceding NoOps on the same engine."""
    nwn = [0]
    for bb in nc.main_func.blocks:
        newl = []
        changed = False
        for inst in bb.instructions:
            si = inst.sync_info
            waits = list(si.on_wait) if (si and si.on_wait) else []
            if len(waits) > 1:
                imm = [w for w in waits if w.wait_reg is None]
                reg = [w for w in waits if w.wait_reg is not None]
                keep = (reg + imm)[:1]
                spill = [w for w in waits if w not in keep]
                assert not any(w.wait_reg is not None for w in spill), inst.name
                for w in spill:
                    nwn[0] += 1
                    nop = mybir.InstNoOp(name=f"I-wsplit-{nwn[0]}", ins=[], outs=[])
                    nop.engine = inst.engine
                    nop.sync_info = mybir.SyncInfo(on_wait=[w], on_update=[])
                    nc.register_instruction(nop)
                    newl.append(nop)
                si.on_wait = keep
                changed = True
            newl.append(inst)
        if changed:
            bb.instructions = newl
    return nc


def _get_program():
    if "nc" not in _CACHE:
        _CACHE["nc"] = _build_program()
    return _CACHE["nc"]


def _host_consts():
    if "consts" in _CACHE:
        return _CACHE["consts"]
    import ml_dtypes
    k = np.arange(128)
    alow = (k[:, None] > k[None, :]).astype(np.float32)      # [k > j]
    uinc = (k[:, None] <= k[None, :]).astype(np.float32)     # [k <= i]
    idn = np.eye(128, dtype=np.float32)
    consts = dict(
        alow=alow.astype(ml_dtypes.bfloat16),
        uinc=uinc.astype(ml_dtypes.bfloat16),
        idnb=idn.astype(ml_dtypes.bfloat16),
        idnf=idn,
        ones=np.ones((128, 1), ml_dtypes.bfloat16),
        onesrf=np.ones((1, 128), np.float32),
        onesrb=np.ones((1, 128), ml_dtypes.bfloat16),
    )
    _CACHE["consts"] = consts
    return consts


def _core_inputs(x_seq, p):
    """x_seq: (L, D_MODEL) f32 (already flipped for bw); p: dict of params."""
    import ml_dtypes
    consts = _host_consts()
    dcol = p["D"].astype(np.float32).repeat(HD).reshape(8, 128).T.copy()
    convw = np.ascontiguousarray(
        p["conv_w"].astype(np.float32).reshape(4, 10, 128).transpose(2, 1, 0)
    )
    convb = np.ascontiguousarray(p["conv_b"].astype(np.float32).reshape(10, 128).T)
    w_out = (p["norm_w"].astype(np.float32)[:, None]
             * p["out_proj"].astype(np.float32))
    return dict(
        xT=np.ascontiguousarray(x_seq.T).astype(ml_dtypes.bfloat16),
        w_in=np.ascontiguousarray(p["in_proj"]).astype(ml_dtypes.bfloat16),
        w_out=np.ascontiguousarray(w_out).astype(ml_dtypes.bfloat16),
        convw=convw,
        convb=convb,
        dtb=p["dt_bias"].astype(np.float32).reshape(16, 1),
        nae=(-np.exp(p["A_log"].astype(np.float32))).reshape(16, 1),
        dcol=dcol,
        **consts,
    )


def kernel(**inputs):
    from concourse.bass_utils import run_bass_kernel_spmd

    nc = _get_program()
    x = np.asarray(inputs["x"], np.float32)
    mask = np.asarray(inputs["padding_mask"])

    def params(pre):
        names = ["in_proj", "conv_w", "conv_b", "dt_bias", "A_log", "D", "norm_w", "out_proj"]
        return {n: np.asarray(inputs[pre + n]) for n in names}

    pf, pb = params("fw_"), params("bw_")
    in_maps = []
    for b in range(B_SZ):
        in_maps.append(_core_inputs(x[b], pf))
    for b in range(B_SZ):
        in_maps.append(_core_inputs(x[b][::-1], pb))

    res = run_bass_kernel_spmd(nc, in_maps, core_ids=list(range(8)))
    out = np.empty((B_SZ, L, D_MODEL), np.float32)
    for b in range(B_SZ):
        yf = res.results[b]["yT"].T
        yb = res.results[B_SZ + b]["yT"].T[::-1]
        out[b] = yf + yb
    out[mask] = 0.0
    return out
